# revision 26
# baseline (speedup 1.0000x reference)
"""Trainium2 Bass kernel for a 2-layer GCN + linear head (SPMD over 8 cores), v2.

Key structure (vs v1): GCN aggregation commutes with the dense weight matmul,
so each layer gathers RAW feature rows and applies W after aggregation:
  h = relu((S^T @ X + dinv^2 * x_own) @ W + b),   S[e,t] = norm_e * (col_e == t)
Layer 1 gathers rows of x directly from a replicated DRAM input (no table
build, no AllGather). Layer 2 AllGathers raw h rows (no matmul before AG).
Aggregation runs transposed (aggT = g^T @ S with g as the stationary operand)
so the post-aggregation W-matmul needs no transpose: lhsT = t2T directly.

Edges are bucketed host-side by (target-window of 128 nodes, source-chunk of
25088 rows) with normalization dinv[row]*w*dinv[col] folded into the S values.
Gather calls cover 3 windows x 1 chunk (<=1920 indices) on a 2048-descriptor
SWDGE ring (dynamic_dma_scratch_size=32768), round-robin over 4 queues.
"""

import os
import sys

sys.path.insert(0, "/opt/trn_rl_repo")

SKIP_MAIN = os.environ.get("KERNEL_SKIP_MAIN") == "1"
SKIP_AG = os.environ.get("KERNEL_SKIP_AG") == "1"
REPS = int(os.environ.get("KERNEL_REPS", "1"))
SCRATCH = int(os.environ.get("KERNEL_SCRATCH", "16384"))
MAX_CALL_TILES = int(os.environ.get("KERNEL_MAX_CALL_TILES", "8"))

import numpy as np
import ml_dtypes

import concourse.bass as bass
import concourse.mybir as mybir
import concourse.tile as tile
from concourse import bacc, library_config
from concourse.bass_utils import run_bass_kernel_spmd

BF16 = mybir.dt.bfloat16
F32 = mybir.dt.float32
I16 = mybir.dt.int16

NCORES = 8
F = 128
NPRED = 16
GSIZE = 4  # windows per gather/matmul group


class Cfg:
    def __init__(self, n_nodes, cap):
        self.n = n_nodes
        self.per = n_nodes // NCORES
        self.nwin = (self.per + 127) // 128
        self.nwpad = self.nwin * 128
        self.nchunk = 4
        self.chspan = ((n_nodes + self.nchunk * 128 - 1) // (self.nchunk * 128)) * 128
        assert self.chspan <= 32768
        self.npad = ((n_nodes + 127) // 128) * 128  # padded rows of x input
        self.cap = cap
        self.tpb = cap // 128
        self.groups = []
        w = 0
        while w < self.nwin:
            g = min(GSIZE, self.nwin - w)
            self.groups.append(list(range(w, w + g)))
            w += g
        # slot layout: for g: for c: for w in g: cap   (one gather call per (g,c))
        self.bucket_base = {}
        pos = 0
        for wl in self.groups:
            for c in range(self.nchunk):
                for w in wl:
                    self.bucket_base[(w, c)] = pos
                    pos += cap
        self.nslot = pos
        self.ntile = pos // 128
        self.max_gt = GSIZE * self.tpb
        assert MAX_CALL_TILES * 128 <= SCRATCH // 16  # SWDGE ring limit per call
        # calls: one per bucket (window, chunk) so each core's per-bucket
        # padding is a trailing run of -1 indices the gather Q7 truncates
        self.calls = []
        self.group_calls = {}  # (gi, c) -> list of (tile_off_in_batch, ntiles)
        for gi, wl in enumerate(self.groups):
            for c in range(self.nchunk):
                s0 = self.bucket_base[(wl[0], c)]
                runs = []
                for wi, w in enumerate(wl):
                    off = wi * self.tpb
                    t = self.tpb
                    o2 = 0
                    while o2 < t:
                        ct = min(MAX_CALL_TILES, t - o2)
                        self.calls.append((s0 + (off + o2) * 128, ct))
                        runs.append((off + o2, ct))
                        o2 += ct
                self.group_calls[(gi, c)] = runs
        # per-call valid-index counts (same across cores; set by prep_inputs)
        self.call_counts = [nt * 128 for _, nt in self.calls]
        self.b1_zero = self.b2_zero = self.bout_zero = False


def build_nc(cfg: Cfg):
    nc = bacc.Bacc(
        "TRN2",
        target_bir_lowering=False,
        num_swdge_queues=4,
        dynamic_dma_scratch_size=SCRATCH,
    )
    per, nwin, nwpad = cfg.per, cfg.nwin, cfg.nwpad
    ntile, nslot = cfg.ntile, cfg.nslot

    # inputs (per core)
    xfull_d = nc.dram_tensor("xfull", [cfg.npad, 128], BF16, kind="ExternalInput")
    xsT_d = nc.dram_tensor("xsT", [128, nwpad], BF16, kind="ExternalInput")
    dinv2_d = nc.dram_tensor("dinv2", [128, nwin], F32, kind="ExternalInput")
    idx_d = nc.dram_tensor("idx", [128, nslot // 16], I16, kind="ExternalInput")
    colv_d = nc.dram_tensor("colv", [128, ntile], BF16, kind="ExternalInput")
    wv_d = nc.dram_tensor("wv", [128, ntile], BF16, kind="ExternalInput")
    w1_d = nc.dram_tensor("w1", [128, 128], BF16, kind="ExternalInput")
    w2_d = nc.dram_tensor("w2", [128, 128], BF16, kind="ExternalInput")
    wout_d = nc.dram_tensor("woutT", [128, NPRED], BF16, kind="ExternalInput")
    b1_d = nc.dram_tensor("b1bc", [128, 128], F32, kind="ExternalInput")
    b2_d = nc.dram_tensor("b2bc", [128, 128], F32, kind="ExternalInput")
    bout_d = nc.dram_tensor("boutbc", [NPRED, 128], F32, kind="ExternalInput")
    iota_d = nc.dram_tensor("iota", [128, cfg.max_gt * 128], BF16, kind="ExternalInput")
    gcnt_d = nc.dram_tensor(
        "gcnt", [1, len(cfg.calls)], mybir.dt.int32, kind="ExternalInput"
    )
    ident_d = nc.dram_tensor("ident", [128, 128], BF16, kind="ExternalInput")

    outT = nc.dram_tensor("outT", [NPRED, nwpad], F32, kind="ExternalOutput")

    # internal DRAM
    table2 = nc.dram_tensor("table2", [cfg.n, 128], BF16, addr_space="Shared")
    ag_in = nc.dram_tensor("ag_in", [per, 128], BF16)

    qctr = [0]
    cctr = [0]

    with tile.TileContext(nc) as tc:
        with (
            tc.tile_pool(name="const", bufs=1) as cp,
            tc.tile_pool(name="big", bufs=1) as bigp,
            tc.tile_pool(name="work", bufs=8) as wp,
            tc.tile_pool(name="gat", bufs=12) as gp,
            tc.tile_pool(name="idxp", bufs=8) as idxp,
            tc.tile_pool(name="ret", bufs=12) as rp,
        ):
            nc.gpsimd.load_library(library_config.mlp)
            iota_t = cp.tile([128, cfg.max_gt * 128], BF16)
            nc.sync.dma_start(iota_t[:], iota_d[:])
            ident_t = cp.tile([128, 128], BF16)
            nc.sync.dma_start(ident_t[:], ident_d[:])
            w1_t = cp.tile([128, 128], BF16)
            nc.sync.dma_start(w1_t[:], w1_d[:])
            w2_t = cp.tile([128, 128], BF16)
            nc.sync.dma_start(w2_t[:], w2_d[:])
            wout_t = cp.tile([128, NPRED], BF16)
            nc.sync.dma_start(wout_t[:], wout_d[:])
            b1_t = cp.tile([128, 128], F32)
            nc.sync.dma_start(b1_t[:], b1_d[:])
            b2_t = cp.tile([128, 128], F32)
            nc.sync.dma_start(b2_t[:], b2_d[:])
            bout_t = cp.tile([NPRED, 128], F32)
            nc.sync.dma_start(bout_t[:], bout_d[:])
            colv_t = cp.tile([128, ntile], BF16)
            nc.sync.dma_start(colv_t[:], colv_d[:])
            wv_t = cp.tile([128, ntile], BF16)
            nc.sync.dma_start(wv_t[:], wv_d[:])
            xsT_t = cp.tile([128, nwpad], BF16)
            nc.sync.dma_start(xsT_t[:], xsT_d[:])
            dinv2_t = cp.tile([128, nwin], F32)
            nc.sync.dma_start(dinv2_t[:], dinv2_d[:])
            gcnt_t = cp.tile([1, len(cfg.calls)], mybir.dt.int32)
            nc.sync.dma_start(gcnt_t[:], gcnt_d[:])
            nreg = nc.gpsimd.alloc_register()

            hsT_t = bigp.tile([128, nwpad], BF16)  # dinv2*h transposed (L2 self)
            if SKIP_MAIN:
                nc.vector.memset(hsT_t[:], 0.0)

            import itertools

            _ctr = itertools.count()

            def s_batch(t0, gt):
                """S tile batch (f-major: S[p, f*gt + k]) for `gt` slot-tiles
                starting at tile index t0. All operands keep the last dim
                packed (k fastest) so DVE runs in 2x mode; the broadcast of
                col/w sits on the middle (f) dim."""
                sb = wp.tile([128, cfg.max_gt * 128], BF16, tag="Sb", name="sb")
                v3 = sb[:, : gt * 128].rearrange("p (f k) -> p f k", k=gt)
                colb = colv_t[:, t0 : t0 + gt][:, None, :].to_broadcast([128, 128, gt])
                wb = wv_t[:, t0 : t0 + gt][:, None, :].to_broadcast([128, 128, gt])
                iot = iota_t[:].rearrange("p (f k) -> p f k", k=cfg.max_gt)[:, :, :gt]
                nc.vector.tensor_tensor(
                    out=v3, in0=iot, in1=colb, op=mybir.AluOpType.is_equal
                )
                nc.vector.tensor_tensor(out=v3, in0=v3, in1=wb, op=mybir.AluOpType.mult)
                return sb

            for _rep in range(REPS):

                def main_pass(src_table, src_rows, layer):
                    """One GCN layer: gather + scatter (transposed) + retire."""
                    with tc.tile_pool(
                        name=f"pm{_rep}{layer}", bufs=4, space="PSUM"
                    ) as pm, tc.tile_pool(
                        name=f"ph{_rep}{layer}", bufs=2, space="PSUM"
                    ) as ph, tc.tile_pool(
                        name=f"pt{_rep}{layer}", bufs=1, space="PSUM"
                    ) as pt, tc.tile_pool(
                        name=f"po{_rep}{layer}", bufs=1, space="PSUM"
                    ) as po:
                        gpos = 0
                        for gi, wl in enumerate(cfg.groups):
                            # one PSUM bank holds all (<=3) window accumulators
                            aggt = pm.tile([128, GSIZE * 128], F32, tag="agg")
                            ps = {
                                w: aggt[:, wi * 128 : (wi + 1) * 128]
                                for wi, w in enumerate(wl)
                            }
                            gtiles = len(wl) * cfg.tpb
                            nidx_g = gtiles * cfg.nchunk * 128
                            idxt = idxp.tile([128, cfg.max_gt * cfg.nchunk * 8], I16, tag="idx")
                            nc.sync.dma_start(
                                idxt[:, : nidx_g // 16],
                                idx_d[:, gpos // 16 : (gpos + nidx_g) // 16],
                            )
                            # gather calls per chunk; 4 SWDGE queues rotate
                            gts = {}
                            ioff = 0
                            for c in range(cfg.nchunk):
                                base = c * cfg.chspan
                                hi = min(base + cfg.chspan, src_rows)
                                gt = gp.tile([128, cfg.max_gt, 128], BF16, tag="g")
                                for off, ct in cfg.group_calls[(gi, c)]:
                                    nidx = ct * 128
                                    ci = cctr[0] % len(cfg.calls)
                                    cctr[0] += 1
                                    nc.gpsimd.load(nreg, gcnt_t[0:1, ci : ci + 1])
                                    nc.gpsimd.dma_gather(
                                        gt[:, off : off + ct, :],
                                        src_table[base:hi, :],
                                        idxt[:, ioff : ioff + nidx // 16],
                                        nidx,
                                        nreg,
                                        128,
                                        single_packet=False,
                                        queue_num=qctr[0] % 4,
                                    )
                                    qctr[0] += 1
                                    ioff += nidx // 16
                                gts[c] = gt
                            sbts = {
                                c: s_batch(
                                    cfg.bucket_base[(wl[0], c)] // 128, gtiles
                                )
                                for c in range(cfg.nchunk)
                            }
                            # aggT[w] += g_tile^T @ S_tile  (g stationary); one
                            # window's accumulation group completes before the
                            # next starts (PSUM groups can't interleave per bank)
                            for wi, w in enumerate(wl):
                                for c in range(cfg.nchunk):
                                    for k in range(cfg.tpb):
                                        j = wi * cfg.tpb + k
                                        sb3 = sbts[c][:, : gtiles * 128].rearrange(
                                            "p (f k) -> p f k", k=gtiles
                                        )
                                        nc.tensor.matmul(
                                            ps[w],
                                            lhsT=gts[c][:, j, :],
                                            rhs=sb3[:, :, j],
                                            start=(c == 0 and k == 0),
                                            stop=(c == cfg.nchunk - 1 and k == cfg.tpb - 1),
                                        )
                            gpos += nidx_g
                            # retire each window of the group
                            for w in wl:
                                cs = slice(w * 128, (w + 1) * 128)
                                # t2T = aggT + selfT  (bf16, feeds W-matmul as lhsT)
                                t2T = rp.tile([128, 128], BF16, tag="t2T")
                                selfT = xsT_t if layer == 1 else hsT_t
                                nc.vector.tensor_tensor(
                                    out=t2T[:],
                                    in0=ps[w],
                                    in1=selfT[:, cs],
                                    op=mybir.AluOpType.add,
                                )
                                wmat = w1_t if layer == 1 else w2_t
                                hp = ph.tile([128, 128], F32, tag="hp")
                                nc.tensor.matmul(
                                    hp[:], lhsT=t2T[:], rhs=wmat[:], start=True, stop=True
                                )
                                bzero = cfg.b1_zero if layer == 1 else cfg.b2_zero
                                if bzero:
                                    relu_in = hp
                                else:
                                    bmat = b1_t if layer == 1 else b2_t
                                    t3 = rp.tile([128, 128], F32, tag="t3")
                                    nc.vector.tensor_tensor(
                                        out=t3[:], in0=hp[:], in1=bmat[:], op=mybir.AluOpType.add
                                    )
                                    relu_in = t3
                                h_sb = rp.tile([128, 128], BF16, tag="hsb")
                                nc.scalar.activation(
                                    out=h_sb[:],
                                    in_=relu_in[:],
                                    func=mybir.ActivationFunctionType.Relu,
                                )
                                if layer == 1:
                                    hi = min((w + 1) * 128, per)
                                    if hi > w * 128:
                                        rows = hi - w * 128
                                        nc.sync.dma_start(
                                            ag_in[w * 128 : hi, :], h_sb[:rows, :]
                                        )
                                    # hsT[:, cs] = (dinv2 * h)^T for L2 self-term
                                    hs = rp.tile([128, 128], BF16, tag="hs")
                                    nc.vector.tensor_scalar(
                                        out=hs[:],
                                        in0=h_sb[:],
                                        scalar1=dinv2_t[:, w : w + 1],
                                        scalar2=None,
                                        op0=mybir.AluOpType.mult,
                                    )
                                    tp = pt.tile([128, 128], BF16, tag="tp")
                                    nc.tensor.transpose(tp[:], hs[:], ident_t[:])
                                    nc.scalar.copy(out=hsT_t[:, cs], in_=tp[:])
                                else:
                                    # head: outT[:, cs] = Wout^T^T @ h2T + bout
                                    tp = pt.tile([128, 128], BF16, tag="tp")
                                    nc.tensor.transpose(tp[:], h_sb[:], ident_t[:])
                                    h2T = rp.tile([128, 128], BF16, tag="h2T")
                                    nc.scalar.copy(out=h2T[:], in_=tp[:])
                                    op = po.tile([NPRED, 128], F32, tag="op")
                                    nc.tensor.matmul(
                                        op[:], lhsT=wout_t[:], rhs=h2T[:], start=True, stop=True
                                    )
                                    ot = rp.tile([NPRED, 128], F32, tag="ot")
                                    nc.vector.tensor_tensor(
                                        out=ot[:], in0=op[:], in1=bout_t[:], op=mybir.AluOpType.add
                                    )
                                    nc.sync.dma_start(outT[:, cs], ot[:])

                if not SKIP_MAIN:
                    main_pass(xfull_d, cfg.n, 1)
                if not SKIP_AG:
                    nc.gpsimd.collective_compute(
                        "AllGather",
                        mybir.AluOpType.bypass,
                        replica_groups=[list(range(NCORES))],
                        ins=[ag_in[:, :]],
                        outs=[table2[:, :]],
                    )
                if not SKIP_MAIN:
                    main_pass(table2, cfg.n, 2)
    nc.compile()
    return nc


def prep_inputs(cfg: Cfg, x, edge_index, edge_weight, W1, b1, W2, b2, Wout, bout):
    per, nwin, nwpad, cap = cfg.per, cfg.nwin, cfg.nwpad, cfg.cap
    row = np.asarray(edge_index[0], dtype=np.int64)
    col = np.asarray(edge_index[1], dtype=np.int64)
    wgt = np.asarray(edge_weight, dtype=np.float64)

    # symmetric normalization folded into edge weights (graph preprocessing)
    cfg.b1_zero = bool(np.all(np.asarray(b1) == 0))
    cfg.b2_zero = bool(np.all(np.asarray(b2) == 0))
    cfg.bout_zero = bool(np.all(np.asarray(bout) == 0))
    deg = np.bincount(col, weights=wgt, minlength=cfg.n) + 1.0
    dinv = 1.0 / np.sqrt(deg)
    wnorm = (dinv[row] * wgt * dinv[col]).astype(np.float32)
    dinv2 = (dinv * dinv).astype(np.float32)

    core = col // per
    col_local = col - core * per
    win = col_local >> 7
    cw = (col_local & 127).astype(np.float32)
    chunk = row // cfg.chspan
    idx16 = (row - chunk * cfg.chspan).astype(np.int16)

    nbuck_core = nwin * cfg.nchunk
    bid = (core * nbuck_core + win * cfg.nchunk + chunk).astype(np.int64)
    order = np.lexsort((idx16, bid))
    bid_s = bid[order]
    counts = np.bincount(bid_s, minlength=NCORES * nbuck_core)
    assert counts.max() <= cap, f"bucket overflow: {counts.max()} > {cap}"
    starts = np.zeros(NCORES * nbuck_core + 1, dtype=np.int64)
    np.cumsum(counts, out=starts[1:])
    rank = np.arange(len(order)) - starts[bid_s]

    base_1core = np.zeros(nbuck_core, dtype=np.int64)
    for (w, c), b in cfg.bucket_base.items():
        base_1core[w * cfg.nchunk + c] = b
    slot = base_1core[bid_s % nbuck_core] + rank
    core_s = bid_s // nbuck_core

    idx_all = np.full((NCORES, cfg.nslot), -1, dtype=np.int16)
    colv = np.zeros((NCORES, cfg.nslot), dtype=np.float32)
    wv = np.zeros((NCORES, cfg.nslot), dtype=np.float32)
    idx_all[core_s, slot] = idx16[order]
    colv[core_s, slot] = cw[order]
    wv[core_s, slot] = wnorm[order]
    # first 3 groups gather row 0 for padding instead of skipping: flushes
    # non-finite garbage out of the 12 reused SBUF gather buffers once
    if len(cfg.groups) > 3:
        flush_end = cfg.bucket_base[(cfg.groups[3][0], 0)]
    else:
        flush_end = cfg.nslot
    head = idx_all[:, :flush_end]
    head[head < 0] = 0

    # per-call per-core real-index counts for num_idxs_reg (the idx tails
    # beyond the count are -1; decode and Q7 then agree on descriptor counts).
    # First 3 groups use the full count (their padding idx is 0, see below).
    cnts3 = counts.reshape(NCORES, nwin, cfg.nchunk)
    gcnt = np.zeros((NCORES, 1, len(cfg.calls)), dtype=np.int32)
    ci = 0
    for gi, wl in enumerate(cfg.groups):
        for c in range(cfg.nchunk):
            for w in wl:
                if gi < 3 or len(cfg.groups) <= 3:
                    gcnt[:, 0, ci] = cfg.cap
                else:
                    gcnt[:, 0, ci] = cnts3[:, w, c]
                ci += 1
    assert ci == len(cfg.calls), (ci, len(cfg.calls))

    # wrap idx per call: [128, nslot/16]
    idx_wrapped = np.zeros((NCORES, 128, cfg.nslot // 16), dtype=np.int16)
    for s0, nt in cfg.calls:
        n = nt * 128
        blk = idx_all[:, s0 : s0 + n].reshape(NCORES, n // 16, 16)
        blk = np.transpose(blk, (0, 2, 1))
        idx_wrapped[:, :, s0 // 16 : (s0 + n) // 16] = np.tile(blk, (1, 8, 1))

    colv_t = np.transpose(colv.reshape(NCORES, cfg.ntile, 128), (0, 2, 1)).astype(
        ml_dtypes.bfloat16
    )
    wv_t = np.transpose(wv.reshape(NCORES, cfg.ntile, 128), (0, 2, 1)).astype(
        ml_dtypes.bfloat16
    )

    xv = np.asarray(x, dtype=np.float32)
    xfull = np.zeros((cfg.npad, 128), dtype=ml_dtypes.bfloat16)
    xfull[: cfg.n] = xv.astype(ml_dtypes.bfloat16)

    # per-core self-term (dinv2 * x)^T and dinv2 tiles
    xsT = np.zeros((NCORES, 128, nwpad), dtype=ml_dtypes.bfloat16)
    dinv2_t = np.zeros((NCORES, 128, nwin), dtype=np.float32)
    for d in range(NCORES):
        xs = np.zeros((nwpad, 128), dtype=np.float32)
        xs[:per] = xv[d * per : (d + 1) * per] * dinv2[d * per : (d + 1) * per, None]
        xsT[d] = np.ascontiguousarray(xs.T).astype(ml_dtypes.bfloat16)
        dl = np.zeros(nwpad, dtype=np.float32)
        dl[:per] = dinv2[d * per : (d + 1) * per]
        dinv2_t[d] = dl.reshape(nwin, 128).T

    iota = np.broadcast_to(
        np.repeat(np.arange(128, dtype=np.float32), cfg.max_gt),
        (128, cfg.max_gt * 128),
    ).astype(ml_dtypes.bfloat16)
    ident = np.eye(128, dtype=np.float32).astype(ml_dtypes.bfloat16)
    common = {
        "xfull": xfull,
        "w1": np.asarray(W1, np.float32).astype(ml_dtypes.bfloat16),
        "w2": np.asarray(W2, np.float32).astype(ml_dtypes.bfloat16),
        "woutT": np.ascontiguousarray(np.asarray(Wout, np.float32).T).astype(
            ml_dtypes.bfloat16
        ),
        "b1bc": np.broadcast_to(np.asarray(b1, np.float32), (128, 128)).copy(),
        "b2bc": np.broadcast_to(np.asarray(b2, np.float32), (128, 128)).copy(),
        "boutbc": np.broadcast_to(
            np.asarray(bout, np.float32)[:, None], (NPRED, 128)
        ).copy(),
        "iota": iota,
        "ident": ident,
    }
    in_maps = []
    for d in range(NCORES):
        m = dict(common)
        m["colv"] = colv_t[d]
        m["wv"] = wv_t[d]
        m["idx"] = idx_wrapped[d]
        m["xsT"] = xsT[d]
        m["dinv2"] = dinv2_t[d]
        m["gcnt"] = gcnt[d]
        in_maps.append(m)
    return in_maps


_CACHE = {}


def run(cfg, x, edge_index, edge_weight, W1, b1, W2, b2, Wout, bout):
    in_maps = prep_inputs(cfg, x, edge_index, edge_weight, W1, b1, W2, b2, Wout, bout)
    key = (cfg.n, cfg.cap, cfg.b1_zero, cfg.b2_zero, cfg.bout_zero)
    if key not in _CACHE:
        _CACHE[key] = build_nc(cfg)
    nc = _CACHE[key]
    res = run_bass_kernel_spmd(nc, in_maps, list(range(NCORES)))
    outs = []
    for d in range(NCORES):
        ot = res.results[d]["outT"]
        outs.append(ot[:, : cfg.per].T)
    return np.ascontiguousarray(np.concatenate(outs, axis=0), dtype=np.float32)


def kernel(x, edge_index, edge_weight, W1, b1, W2, b2, Wout, bout):
    cfg = Cfg(100000, 640)
    return run(cfg, x, edge_index, edge_weight, W1, b1, W2, b2, Wout, bout)


if __name__ == "__main__":
    rng = np.random.default_rng(0)
    n, e = 4096, 65536
    x = rng.standard_normal((n, 128)).astype(np.float32)
    ei = rng.integers(0, n, (2, e)).astype(np.int64)
    ew = rng.random(e).astype(np.float32)
    W1 = (rng.standard_normal((128, 128)) / np.sqrt(128)).astype(np.float32)
    W2 = (rng.standard_normal((128, 128)) / np.sqrt(128)).astype(np.float32)
    Wout = (rng.standard_normal((16, 128)) / np.sqrt(128)).astype(np.float32)
    b1 = np.zeros(128, np.float32)
    b2 = np.zeros(128, np.float32)
    bout = np.zeros(16, np.float32)

    def gcn(xx, W, b):
        deg = np.bincount(ei[1], weights=ew, minlength=n) + 1.0
        dinv = 1.0 / np.sqrt(deg)
        xw = xx @ W
        msg = xw[ei[0]] * (dinv[ei[0]] * ew * dinv[ei[1]])[:, None]
        out = np.zeros_like(xw)
        np.add.at(out, ei[1], msg)
        out += xw * (dinv**2)[:, None]
        return np.maximum(out + b, 0.0)

    h = gcn(x, W1, b1)
    h = gcn(h, W2, b2)
    ref = h @ Wout.T + bout

    cfg = Cfg(n, 640)
    got = run(cfg, x, ei, ew, W1, b1, W2, b2, Wout, bout)
    err = np.abs(got - ref).max() / (np.abs(ref).max() + 1e-9)
    l2 = np.linalg.norm(got - ref) / np.linalg.norm(ref)
    print(f"SMOKE: max rel err {err:.3e}   l2 rel {l2:.3e}")

